# revision 17
# baseline (speedup 1.0000x reference)
"""Distributed multi-head attention kernel for 8 TRN2 NeuronCores.

Problem: B=2, N=2048, C=768, H=12 heads of dim 64.
  q = x @ Wq.T ; k = x @ Wk.T ; v = x @ Wv.T      (per-head split)
  out = softmax(q k^T / 8) v                        (full N^2 attention)
  y = concat_heads(out) @ Wo.T + bo

Sharding: 24 (batch, head) pairs -> 3 per core.  Core i owns batch i//4 and
heads 3*(i%4)..3*(i%4)+2.  Projections + attention are fully local (weights
row-sliced on the host).  An 8-way AllToAll then redistributes the per-head
context so core i owns query rows 256*i..256*(i+1) of BOTH batches with all
12 heads, after which the output projection (full Wo, replicated) produces a
disjoint output slice per core.  All matmuls run in bf16 (f32 PSUM accum).
"""

import numpy as np
import ml_dtypes

import concourse.bass as bass
import concourse.mybir as mybir
import concourse.tile as tile
from concourse import bacc
from concourse.bass_utils import run_bass_kernel_spmd
from concourse.masks import make_identity

B, N, C, H, HD = 2, 2048, 768, 12, 64
SCALE = HD ** -0.5          # 0.125
P = 128
CB = C // P                 # 6 contraction blocks of 128 over channels
KB = N // P                 # 16 key blocks
QCH = 512                   # query chunk (max moving free dim)
NQC = N // QCH              # 4
HPC = 3                     # heads per core
NCORES = 8
VW = HPC * (HD + 1)         # 195: v columns per key-block (3 heads + ones col)
RQ = N // NCORES            # 256 query rows per core per batch after A2A

f32 = mybir.dt.float32
bf16 = mybir.dt.bfloat16
Exp = mybir.ActivationFunctionType.Exp
Identity = mybir.ActivationFunctionType.Identity

# head -> (block, partition offset) inside qT_sb / kT_sb [128, 2*2048].
# Identical offsets for q and k per head (PE needs matching base partitions):
# block 0 rows 0:64 = head 0, rows 64:128 = head 1; block 1 rows 0:64 = head 2.
HOFF = {0: (0, 0), 1: (0, 64), 2: (1, 0)}
# wqkT host column order: [q0 q1 | k0 k1 | q2 | k2] (projection passes)
PROJ_PASSES = [
    # (wqkT col offset, M, dest 'q' or 'k', dest block)
    (0, 128, "q", 0),
    (128, 128, "k", 0),
    (256, 64, "q", 1),
    (320, 64, "k", 1),
]


def _body(nc, tc, xT, wqkT, wvT, woT, bo_d, out_d, dbg=None):
    with (
        tc.tile_pool(name="const", bufs=1) as constp,
        tc.tile_pool(name="big", bufs=1) as bigp,
        tc.tile_pool(name="esp", bufs=32) as esp,
        tc.tile_pool(name="smallp", bufs=4) as smallp,
        tc.tile_pool(name="outp", bufs=2) as outp,
        tc.tile_pool(name="psA", bufs=2, space="PSUM") as psA,
        tc.tile_pool(name="psS", bufs=2, space="PSUM") as psS,
        tc.tile_pool(name="psC", bufs=2, space="PSUM") as psC,
        tc.tile_pool(name="psT", bufs=2, space="PSUM") as psT,
        tc.tile_pool(name="dram", bufs=1, space="DRAM") as dramp,
    ):
        # ---- load inputs to SBUF (all bf16 except bias) ----
        xT_sb = bigp.tile([P, CB * N], bf16, name="xT_sb")
        wqkT_sb = bigp.tile([P, CB * 384], bf16, name="wqkT_sb")
        wvT_sb = bigp.tile([P, CB * 192], bf16, name="wvT_sb")
        woT_sb = bigp.tile([P, CB * C], bf16, name="woT_sb")
        bo_sb = bigp.tile([P, CB], f32, name="bo_sb")
        ident = constp.tile([P, P], bf16, name="ident")
        for cb in range(CB):
            nc.sync.dma_start(xT_sb[:, cb * N:(cb + 1) * N], xT[cb * P:(cb + 1) * P, :])
            nc.sync.dma_start(wqkT_sb[:, cb * 384:(cb + 1) * 384], wqkT[cb * P:(cb + 1) * P, :])
            nc.sync.dma_start(wvT_sb[:, cb * 192:(cb + 1) * 192], wvT[cb * P:(cb + 1) * P, :])
            nc.sync.dma_start(woT_sb[:, cb * C:(cb + 1) * C], woT[cb * P:(cb + 1) * P, :])
            nc.sync.dma_start(bo_sb[:, cb:cb + 1], bo_d[cb * P:(cb + 1) * P, :])
        make_identity(nc, ident)

        # ---- Q/K projections into q_T / k_T [head-dim on partitions] ----
        qT_sb = bigp.tile([P, 2 * N], bf16, name="qT_sb")
        kT_sb = bigp.tile([P, 2 * N], bf16, name="kT_sb")
        for co, m, dst, blk in PROJ_PASSES:
            dst_sb = qT_sb if dst == "q" else kT_sb
            for qn in range(NQC):
                ps = psA.tile([P, QCH], f32, name=f"pj_{dst}_{blk}_{qn}", tag="psA")
                for cb in range(CB):
                    nc.tensor.matmul(
                        ps[:m, :],
                        lhsT=wqkT_sb[:, cb * 384 + co: cb * 384 + co + m],
                        rhs=xT_sb[:, cb * N + qn * QCH: cb * N + qn * QCH + QCH],
                        start=(cb == 0), stop=(cb == CB - 1),
                    )
                nc.vector.tensor_copy(
                    dst_sb[:m, blk * N + qn * QCH: blk * N + qn * QCH + QCH], ps[:m, :])

        # ---- V projection into [n, 3*(64+1)] layout with ones columns ----
        # NB: start=True clears has_written for the WHOLE psum bank, so each
        # bank may hold exactly one accumulation group: project all 3 heads
        # as one [128, 192] group, then split into the 65-strided layout.
        v_sb = bigp.tile([P, KB * VW], bf16, name="v_sb")
        for nb in range(KB):
            ps = psA.tile([P, QCH], f32, name=f"vps_{nb}", tag="psA")
            for cb in range(CB):
                nc.tensor.matmul(
                    ps[:, 0:192],
                    lhsT=xT_sb[:, cb * N + nb * P: cb * N + (nb + 1) * P],
                    rhs=wvT_sb[:, cb * 192:(cb + 1) * 192],
                    start=(cb == 0), stop=(cb == CB - 1),
                )
            for h in range(HPC):
                nc.vector.tensor_copy(v_sb[:, nb * VW + h * 65: nb * VW + h * 65 + 64],
                                      ps[:, h * 64:(h + 1) * 64])
                nc.vector.memset(v_sb[:, nb * VW + h * 65 + 64: nb * VW + h * 65 + 65], 1.0)

        # ---- attention: per (head, q-chunk); scores kept on-chip ----
        # Phase A(t): 16 score matmuls + exps for q-chunk t (all es tiles kept).
        # Phase B(t): per 128-row q-block, one single-group PV accumulation in
        # its own psum bank (start=True clears the whole bank), + normalize.
        # Emission is skewed (A(t+1) before B(t)) so ACT exps overlap PE's PV.
        ctx_sb = bigp.tile([P, KB * 192], bf16, name="ctx_sb")  # [qblock][head*64]
        es_lists = {}

        def attn_a(t):
            h, qc = divmod(t, NQC)
            hb_, ho_ = HOFF[h]
            es_tiles = []
            for kb in range(KB):
                sps = psS.tile([P, QCH], f32, name=f"sps_{t}_{kb}", tag="psS")
                nc.tensor.matmul(
                    sps[:, :],
                    lhsT=kT_sb[ho_:ho_ + 64, hb_ * N + kb * P: hb_ * N + (kb + 1) * P],
                    rhs=qT_sb[ho_:ho_ + 64, hb_ * N + qc * QCH: hb_ * N + qc * QCH + QCH],
                    start=True, stop=True,
                )
                es = esp.tile([P, QCH], bf16, name=f"es_{t}_{kb}", tag="es")
                nc.scalar.activation(es, sps, Exp, scale=SCALE)
                es_tiles.append(es)
            es_lists[t] = es_tiles

        def attn_b(t):
            h, qc = divmod(t, NQC)
            es_tiles = es_lists.pop(t)
            for qb in range(4):
                cps = psC.tile([P, 65], f32, name=f"cps_{t}_{qb}", tag="psC")
                for kb in range(KB):
                    nc.tensor.matmul(
                        cps[:, :],
                        lhsT=es_tiles[kb][:, qb * P:(qb + 1) * P],
                        rhs=v_sb[:, kb * VW + h * 65: kb * VW + (h + 1) * 65],
                        start=(kb == 0), stop=(kb == KB - 1),
                    )
                rec = smallp.tile([P, 1], f32, name=f"rec_{t}_{qb}", tag="rec")
                nc.vector.reciprocal(rec, cps[:, 64:65])
                qg = qc * 4 + qb
                nc.vector.tensor_scalar_mul(
                    ctx_sb[:, qg * 192 + h * 64: qg * 192 + (h + 1) * 64],
                    cps[:, 0:64],
                    rec,
                )

        NT = HPC * NQC
        for t in range(NT):
            attn_a(t)
            if t > 0:
                attn_b(t - 1)
        attn_b(NT - 1)

        # ---- transpose ctx -> ctx_T [192 j, 2048 q] (2 partition blocks) ----
        ctxT_sb = bigp.tile([P, 2 * N], bf16, name="ctxT_sb")
        for qg in range(KB):
            for h in range(HPC):
                blk, ro = divmod(h * 64, P)
                tp = psT.tile([P, P], bf16, name=f"tp_{qg}_{h}", tag="psT")
                nc.tensor.transpose(tp[ro:ro + 64, :], ctx_sb[:, qg * 192 + h * 64: qg * 192 + (h + 1) * 64], ident)
                nc.vector.tensor_copy(ctxT_sb[ro:ro + 64, blk * N + qg * P: blk * N + (qg + 1) * P],
                                      tp[ro:ro + 64, :])

        # ---- AllToAll: shard q-rows 8 ways; gather all heads of my q-slice ----
        send = dramp.tile([NCORES, 192, RQ], bf16, name="send")
        recv = dramp.tile([NCORES, 192, RQ], bf16, name="recv")
        for j in range(NCORES):
            nc.sync.dma_start(send[j, 0:P, :], ctxT_sb[:, j * RQ:(j + 1) * RQ])
            nc.sync.dma_start(send[j, P:192, :], ctxT_sb[0:64, N + j * RQ: N + (j + 1) * RQ])
        nc.gpsimd.collective_compute(
            "AllToAll", mybir.AluOpType.bypass,
            replica_groups=[list(range(NCORES))],
            ins=[send.opt()], outs=[recv.opt()],
        )
        # recv[s] = heads 3*(s%4)..+2 of batch s//4 for my q rows.
        ctxTf_sb = bigp.tile([P, CB * 2 * RQ], bf16, name="ctxTf_sb")
        rflat = recv.rearrange("s j q -> (s j) q")
        for jb in range(CB):
            nc.sync.dma_start(ctxTf_sb[:, jb * 2 * RQ: jb * 2 * RQ + RQ],
                              rflat[jb * P:(jb + 1) * P, :])
            nc.sync.dma_start(ctxTf_sb[:, jb * 2 * RQ + RQ: (jb + 1) * 2 * RQ],
                              rflat[4 * 192 + jb * P: 4 * 192 + (jb + 1) * P, :])

        if dbg is not None:
            nc.sync.dma_start(dbg["ctx"][:, :], ctx_sb[:, :])
            nc.sync.dma_start(dbg["ctxT"][:, :], ctxT_sb[:, :])
            nc.sync.dma_start(dbg["qT"][:, :], qT_sb[:, :])
            nc.sync.dma_start(dbg["kT"][:, :], kT_sb[:, :])
            nc.sync.dma_start(dbg["v"][:, :], v_sb[:, :])
            nc.sync.dma_start(dbg["recv"][:, :], recv.rearrange("s j q -> (s j) q"))

        # ---- output projection (full Wo) + bias; out_T [c, 2*256] ----
        for cbo in range(CB):
            ps = psA.tile([P, QCH], f32, name=f"ops_{cbo}", tag="psA")
            for jc in range(CB):
                nc.tensor.matmul(
                    ps[:, :],
                    lhsT=woT_sb[:, jc * C + cbo * P: jc * C + (cbo + 1) * P],
                    rhs=ctxTf_sb[:, jc * 2 * RQ:(jc + 1) * 2 * RQ],
                    start=(jc == 0), stop=(jc == CB - 1),
                )
            osb = outp.tile([P, 2 * RQ], f32, name=f"osb_{cbo}", tag="osb")
            nc.scalar.activation(osb, ps, Identity, bias=bo_sb[:, cbo:cbo + 1])
            nc.sync.dma_start(out_d[cbo * P:(cbo + 1) * P, :], osb)


def build(debug_outs=False):
    nc = bacc.Bacc("TRN2", target_bir_lowering=False, debug=False, num_devices=NCORES)
    xT = nc.dram_tensor("xT", [C, N], bf16, kind="ExternalInput").ap()
    wqkT = nc.dram_tensor("wqkT", [C, 2 * HPC * HD], bf16, kind="ExternalInput").ap()
    wvT = nc.dram_tensor("wvT", [C, HPC * HD], bf16, kind="ExternalInput").ap()
    woT = nc.dram_tensor("woT", [C, C], bf16, kind="ExternalInput").ap()
    bo_d = nc.dram_tensor("bo", [C, 1], f32, kind="ExternalInput").ap()
    out_d = nc.dram_tensor("out", [C, 2 * RQ], f32, kind="ExternalOutput").ap()
    dbg = None
    if debug_outs:
        dbg = {
            "ctx": nc.dram_tensor("dbg_ctx", [P, KB * 192], bf16, kind="ExternalOutput").ap(),
            "ctxT": nc.dram_tensor("dbg_ctxT", [P, 2 * N], bf16, kind="ExternalOutput").ap(),
            "qT": nc.dram_tensor("dbg_qT", [P, 2 * N], bf16, kind="ExternalOutput").ap(),
            "kT": nc.dram_tensor("dbg_kT", [P, 2 * N], bf16, kind="ExternalOutput").ap(),
            "v": nc.dram_tensor("dbg_v", [P, KB * VW], bf16, kind="ExternalOutput").ap(),
            "recv": nc.dram_tensor("dbg_recv", [NCORES * 192, RQ], bf16, kind="ExternalOutput").ap(),
        }
    with tile.TileContext(nc) as tc:
        _body(nc, tc, xT, wqkT, wvT, woT, bo_d, out_d, dbg)
    nc.compile()
    return nc


_NC = None


def _get_nc():
    global _NC
    if _NC is None:
        _NC = build()
    return _NC


def make_in_maps(x, Wq, Wk, Wv, Wo, bo):
    x = np.asarray(x, np.float32)
    woT = np.ascontiguousarray(np.asarray(Wo, np.float32).T).astype(ml_dtypes.bfloat16)
    bo_col = np.ascontiguousarray(np.asarray(bo, np.float32).reshape(C, 1))
    in_maps = []
    for i in range(NCORES):
        b = i // 4
        hs = (i % 4) * HPC
        rq = slice(hs * HD, (hs + HPC) * HD)
        wq_s = np.asarray(Wq, np.float32)[rq]  # [192, 768]
        wk_s = np.asarray(Wk, np.float32)[rq]
        # column order matches PROJ_PASSES: [q0 q1 | k0 k1 | q2 | k2]
        wqk = np.concatenate([wq_s[0:128], wk_s[0:128], wq_s[128:192], wk_s[128:192]], axis=0).T
        in_maps.append({
            "xT": np.ascontiguousarray(x[b].T).astype(ml_dtypes.bfloat16),
            "wqkT": np.ascontiguousarray(wqk).astype(ml_dtypes.bfloat16),
            "wvT": np.ascontiguousarray(np.asarray(Wv, np.float32)[rq].T).astype(ml_dtypes.bfloat16),
            "woT": woT,
            "bo": bo_col,
        })
    return in_maps


def unshard(results):
    out = np.empty((B, N, C), np.float32)
    for i, r in enumerate(results):
        o = r["out"]  # [768, 512]: cols 0-255 batch 0, 256-511 batch 1
        out[0, i * RQ:(i + 1) * RQ, :] = o[:, :RQ].T
        out[1, i * RQ:(i + 1) * RQ, :] = o[:, RQ:].T
    return out


def kernel(x, Wq, Wk, Wv, Wo, bo):
    nc = _get_nc()
    in_maps = make_in_maps(x, Wq, Wk, Wv, Wo, bo)
    res = run_bass_kernel_spmd(nc, in_maps, core_ids=list(range(NCORES)))
    return unshard(res.results)
